# revision 13
# baseline (speedup 1.0000x reference)
"""Trainium2 Bass kernel for a fused transformer block (attention + FF).

Computation (B=2, T=2048, E=1024, H=16 heads, dh=64):
    q,k,v = x@Wq, x@Wk, x@Wv          (per-head, no bias)
    scores = q k^T / sqrt(E)  (causal)
    attn   = softmax(scores) v
    x1     = x + attn                  (no out-projection)
    out    = x1 + relu(x1 @ Wf + bf)

Sharding across 8 NeuronCores:
  - Attention is head-parallel: core c owns heads {2c, 2c+1} (128 feature
    cols of q/k/v).  Each core computes qT/kT/vT for its heads over ALL
    tokens, then causal-softmax attention in a transposed (S^T) layout.
  - An AllToAll redistributes attn^T from head-sharded to token-sharded.
  - FF is token-parallel: core c owns B*T/8 tokens of the flattened [B*T].

Performance structure (v2):
  - All inputs are pre-cast to bf16 on the host (halves HBM traffic).
  - xT is loaded in 512-token slabs alternating between the HWDGE (sync)
    and SWDGE (gpsimd) DMA queues so QKV matmuls start ~6us in.
  - Scores for BOTH heads go into one [128, 1024] PSUM tile (2 banks) as
    two concurrent K=64 matmuls row-packed via tile_position (0,0)/(64,0).
    st tiles are double-buffered (4 banks) so exp (ScalarE) overlaps the
    next score matmuls; AV matmuls are software-pipelined one kc behind.
  - Only the 128-wide diagonal block needs the triangular mask; columns
    left of it are memset and skipped by exp.
  - v natural layout comes from vT (N=512 matmuls) + PE-transpose chunks.
  - Softmax normalization happens on the FF side after the AllToAll; the
    denominators ride along as v's appended ones-column (row 64 of AV).
  - Batch 1's QKV work is emitted interleaved into batch 0's attention so
    the PE stays busy while the scalar engine works through exp calls.
"""

import math

import numpy as np

B, T, E, H = 2, 2048, 1024, 16
DH = E // H
NCORES = 8
P = 128  # partitions
QI_BLK = 512  # qi block width (one PSUM bank of fp32)
ECH = E // P  # feature chunks of 128
TOK_PC = B * T // NCORES  # tokens per core (FF phase)
TPH = TOK_PC // B  # tokens per core per batch
N_KC = T // P  # kj chunks per batch
N_QI = T // QI_BLK


def _build_bass(n_cores: int = NCORES):
    from contextlib import ExitStack

    import concourse.bacc as bacc
    import concourse.mybir as mybir
    import concourse.tile as tile

    t_all = B * T

    nc = bacc.Bacc(
        "TRN2",
        target_bir_lowering=False,
        debug=False,
        num_devices=n_cores,
    )

    dt = mybir.dt
    f32, bf16 = dt.float32, dt.bfloat16
    AF = mybir.ActivationFunctionType

    # ---- I/O (all matmul operands pre-cast to bf16 on the host) --------
    xT = nc.dram_tensor("xT", [E, t_all], bf16, kind="ExternalInput")
    xsliceT = nc.dram_tensor("xsliceT", [E, TOK_PC], bf16, kind="ExternalInput")
    wqkv = nc.dram_tensor("wqkv", [E, 3 * P], bf16, kind="ExternalInput")
    wf = nc.dram_tensor("wf", [E, E], bf16, kind="ExternalInput")
    bfcol = nc.dram_tensor("bfcol", [E, 1], f32, kind="ExternalInput")
    tri = nc.dram_tensor("tri", [P, P], bf16, kind="ExternalInput")
    ident = nc.dram_tensor("ident", [P, P], bf16, kind="ExternalInput")
    indmat = nc.dram_tensor("indmat", [2 * ECH, ECH, P], bf16, kind="ExternalInput")
    outT = nc.dram_tensor("outT", [E, TOK_PC], f32, kind="ExternalOutput")

    # Internal DRAM: per-batch all-to-all buffers
    # ([128 attn rows | 2 den rows] per shard); one exchange per batch so
    # batch 0's exchange overlaps batch 1's attention compute.
    a2a_in = [
        nc.dram_tensor(f"a2a_in{b}", [n_cores, P + 2, TPH], f32) for b in range(B)
    ]
    a2a_out = [
        nc.dram_tensor(f"a2a_out{b}", [n_cores, P + 2, TPH], f32) for b in range(B)
    ]

    scale = 1.0 / math.sqrt(E)

    with tile.TileContext(nc) as tc, ExitStack() as ctx:
        # ---- persistent SBUF -------------------------------------------
        persist = ctx.enter_context(tc.tile_pool(name="persist", bufs=1))
        wqkv_sb = persist.tile([P, ECH, 3 * P], bf16)
        wf_sb = persist.tile([P, ECH, ECH, P], bf16)
        bf_sb = persist.tile([P, ECH], f32)
        tri_sb = persist.tile([P, P], bf16)
        id_sb = persist.tile([P, P], bf16)
        ind_sb = persist.tile([2 * ECH, ECH, P], bf16)
        xsl_sb = persist.tile([P, ECH, TOK_PC], bf16)

        attn_pool = ctx.enter_context(tc.tile_pool(name="attn_pool", bufs=1))
        attnT = [
            attn_pool.tile([P, T], f32, name=f"attnT{b}", tag=f"attnT{b}")
            for b in range(B)
        ]
        # dens at partitions 0 and 32 (engine writes must be 32-aligned)
        denT = [
            attn_pool.tile([33, T], f32, name=f"denT{b}", tag=f"denT{b}")
            for b in range(B)
        ]

        # ---- PSUM pools: ps(2) + st(2x2) + av0/av1(2) = 8 banks --------
        ps_pool = ctx.enter_context(tc.tile_pool(name="ps", bufs=2, space="PSUM"))
        st_pool = ctx.enter_context(tc.tile_pool(name="st", bufs=2, space="PSUM"))
        av_pool = ctx.enter_context(tc.tile_pool(name="av", bufs=1, space="PSUM"))

        qkv_pool = ctx.enter_context(tc.tile_pool(name="qkv_pool", bufs=2))
        pt_pool = ctx.enter_context(tc.tile_pool(name="pt_pool", bufs=3))

        xt_pool = ctx.enter_context(tc.tile_pool(name="xt_pool", bufs=1))
        xT_sb = xt_pool.tile([P, ECH, t_all], bf16)

        # ---- DMA schedule: xT slabs first, FF-phase tensors later ------
        xT_r = xT.rearrange("(c p) t -> p c t", p=P)
        nc.sync.dma_start(out=wqkv_sb, in_=wqkv.rearrange("(c p) m -> p c m", p=P))
        nc.sync.dma_start(
            out=xT_sb[:, :, 0:QI_BLK], in_=xT_r[:, :, 0:QI_BLK]
        )
        nc.sync.dma_start(out=tri_sb, in_=tri[:, :])
        nc.sync.dma_start(out=id_sb, in_=ident[:, :])
        nc.sync.dma_start(out=bf_sb, in_=bfcol.rearrange("(c p) 1 -> p c", p=P))
        for s in range(1, t_all // QI_BLK):
            sl = slice(s * QI_BLK, (s + 1) * QI_BLK)
            eng = nc.gpsimd if s % 2 else nc.sync
            eng.dma_start(out=xT_sb[:, :, sl], in_=xT_r[:, :, sl])
        nc.gpsimd.dma_start(
            out=wf_sb, in_=wf.rearrange("(c p) (f m) -> p c f m", p=P, m=P)
        )
        nc.gpsimd.dma_start(out=ind_sb, in_=indmat[:, :, :])
        nc.sync.dma_start(
            out=xsl_sb, in_=xsliceT.rearrange("(c p) t -> p c t", p=P)
        )

        qkvs = {}

        def emit_qkv_tb(b, tb):
            """QKV projection chains + v transposes for one 512-token block."""
            qT, kT, vT, v_sb = qkvs[b]
            t0 = b * T
            for which, dst in ((0, qT), (1, kT), (2, vT)):
                ps = ps_pool.tile([P, QI_BLK], f32, name="qk_ps", tag="ps")
                for ec in range(ECH):
                    nc.tensor.matmul(
                        ps,
                        lhsT=wqkv_sb[:, ec, which * P : (which + 1) * P],
                        rhs=xT_sb[
                            :, ec, t0 + tb * QI_BLK : t0 + (tb + 1) * QI_BLK
                        ],
                        start=(ec == 0),
                        stop=(ec == ECH - 1),
                    )
                nc.vector.tensor_copy(
                    dst[:, tb * QI_BLK : (tb + 1) * QI_BLK], ps
                )
            # v natural layout [token, head, dh+1] via PE transpose
            for kc in range(tb * 4, tb * 4 + 4):
                tp = ps_pool.tile([P, P], bf16, name="tp_ps", tag="ps")
                nc.tensor.transpose(tp, vT[:, kc * P : (kc + 1) * P], id_sb)
                nc.vector.tensor_copy(
                    v_sb[:, kc, :, 0:DH], tp.rearrange("p (h d) -> p h d", h=2)
                )
                nc.vector.memset(v_sb[:, kc, :, DH : DH + 1], 1.0)

        def emit_attn_qb(b, qb):
            """Causal attention for one 512-wide query block, both heads."""
            qT, kT, vT, v_sb = qkvs[b]
            last = 4 * qb + 3
            qsl = slice(qb * QI_BLK, (qb + 1) * QI_BLK)
            av0 = av_pool.tile([DH + 1, QI_BLK], f32, name="av0", tag="av0")
            av1 = av_pool.tile([DH + 1, QI_BLK], f32, name="av1", tag="av1")
            pending = None
            for kc in range(last + 1):
                st = st_pool.tile([P, 2 * QI_BLK], f32, name="st", tag="st")
                nc.tensor.matmul(
                    st[:, 0:QI_BLK],
                    lhsT=kT[0:DH, kc * P : (kc + 1) * P],
                    rhs=qT[0:DH, qsl],
                    start=True,
                    stop=True,
                )
                nc.tensor.matmul(
                    st[:, QI_BLK : 2 * QI_BLK],
                    lhsT=kT[DH:P, kc * P : (kc + 1) * P],
                    rhs=qT[DH:P, qsl],
                    start=True,
                    stop=True,
                )
                pt = pt_pool.tile([P, 2 * QI_BLK], bf16, name="pt", tag="pt")
                diag = kc >= 4 * qb
                c0 = (kc - 4 * qb) * P if diag else 0
                if c0 == 0:
                    nc.scalar.activation(pt, st, AF.Exp, scale=scale)
                else:
                    # one 3D-AP instruction covers both heads' regions
                    ptv = pt.rearrange("p (h q) -> p h q", h=2)
                    stv = st.rearrange("p (h q) -> p h q", h=2)
                    nc.vector.memset(ptv[:, :, 0:c0], 0.0)
                    nc.scalar.activation(
                        ptv[:, :, c0:QI_BLK],
                        stv[:, :, c0:QI_BLK],
                        AF.Exp,
                        scale=scale,
                    )
                if diag:
                    nc.vector.tensor_mul(
                        pt[:, c0 : c0 + P], pt[:, c0 : c0 + P], tri_sb
                    )
                    nc.vector.tensor_mul(
                        pt[:, QI_BLK + c0 : QI_BLK + c0 + P],
                        pt[:, QI_BLK + c0 : QI_BLK + c0 + P],
                        tri_sb,
                    )
                if pending is not None:
                    pending()
                kc_ = kc

                def do_av(kc=kc_, pt=pt):
                    nc.tensor.matmul(
                        av0,
                        lhsT=v_sb[:, kc, 0, :],
                        rhs=pt[:, 0:QI_BLK],
                        start=(kc == 0),
                        stop=(kc == last),
                    )
                    nc.tensor.matmul(
                        av1,
                        lhsT=v_sb[:, kc, 1, :],
                        rhs=pt[:, QI_BLK : 2 * QI_BLK],
                        start=(kc == 0),
                        stop=(kc == last),
                    )

                pending = do_av
            pending()
            # spill unnormalized numerators + denominators
            nc.vector.tensor_copy(attnT[b][0:DH, qsl], av0[0:DH, :])
            nc.vector.tensor_copy(attnT[b][DH:P, qsl], av1[0:DH, :])
            nc.vector.tensor_copy(denT[b][0:1, qsl], av0[DH : DH + 1, :])
            nc.vector.tensor_copy(denT[b][32:33, qsl], av1[DH : DH + 1, :])

        def emit_a2a(b):
            nc.sync.dma_start(
                out=a2a_in[b].rearrange("s p t -> p s t")[0:P],
                in_=attnT[b].rearrange("p (s t) -> p s t", s=n_cores),
            )
            nc.gpsimd.dma_start(
                out=a2a_in[b].rearrange("s p t -> p s t")[P : P + 2],
                in_=denT[b][0:33:32, :].rearrange("p (s t) -> p s t", s=n_cores),
            )
            nc.gpsimd.collective_compute(
                "AllToAll",
                mybir.AluOpType.bypass,
                replica_groups=[list(range(n_cores))],
                ins=[a2a_in[b][:].opt()],
                outs=[a2a_out[b][:].opt()],
            )

        # ---- FF emitters (token-parallel: TPH tokens per batch) --------
        ff_state = {}

        def emit_ff_norm(bh, ff_pool, ff_work):
            """Softmax-normalize + residual for this bh's tokens."""
            den16, recip16, recip_bf, xres_bf = ff_state["tiles"]
            hsl = slice(bh * TPH, (bh + 1) * TPH)
            a2a_r = a2a_out[bh].rearrange("s p t -> p s t")
            nc.gpsimd.dma_start(out=den16[:, hsl], in_=a2a_r[P : P + 2])
            nc.vector.reciprocal(recip16[:, hsl], den16[:, hsl])
            nc.vector.tensor_copy(recip_bf[:, hsl], recip16[:, hsl])
            at_all = ff_work.tile([P, ECH, TPH], f32, name="at_all", tag="at")
            nc.gpsimd.dma_start(out=at_all, in_=a2a_r[0:P])
            for ec in range(ECH):
                bc = ps_pool.tile([P, TPH], f32, name="bc_ps", tag="ps")
                nc.tensor.matmul(
                    bc,
                    lhsT=ind_sb[:, ec, :],
                    rhs=recip_bf[:, hsl],
                    start=True,
                    stop=True,
                )
                nc.vector.tensor_mul(at_all[:, ec, :], at_all[:, ec, :], bc)
                nc.vector.tensor_add(
                    xres_bf[:, ec, hsl], xsl_sb[:, ec, hsl], at_all[:, ec, :]
                )

        def emit_ff_mm(bh, ff_work):
            den16, recip16, recip_bf, xres_bf = ff_state["tiles"]
            hsl = slice(bh * TPH, (bh + 1) * TPH)
            o_all = ff_work.tile([P, ECH, TPH], f32, name="o_all", tag="o")
            for fc in range(ECH):
                ps = ps_pool.tile([P, TPH], f32, name="ff_ps", tag="ps")
                for ec in range(ECH):
                    nc.tensor.matmul(
                        ps,
                        lhsT=wf_sb[:, ec, fc, :],
                        rhs=xres_bf[:, ec, hsl],
                        start=(ec == 0),
                        stop=(ec == ECH - 1),
                    )
                relu = ff_work.tile([P, TPH], f32, name="relu", tag="relu")
                nc.scalar.activation(
                    relu, ps, AF.Relu, bias=bf_sb[:, fc : fc + 1]
                )
                nc.vector.tensor_add(o_all[:, fc, :], relu, xres_bf[:, fc, hsl])
            nc.sync.dma_start(
                out=outT.rearrange("(c p) t -> p c t", p=P)[:, :, hsl],
                in_=o_all,
            )

        # ---- emission: qkv/attn interleaved; FF(bh0) woven into b1 -----
        for b in range(B):
            qkvs[b] = (
                qkv_pool.tile([P, T], bf16, name=f"qT{b}", tag="qT"),
                qkv_pool.tile([P, T], bf16, name=f"kT{b}", tag="kT"),
                qkv_pool.tile([P, T], bf16, name=f"vT{b}", tag="vT"),
                qkv_pool.tile([P, N_KC, 2, DH + 1], bf16, name=f"v{b}", tag="v"),
            )
        for i in range(N_QI):
            emit_qkv_tb(0, i)
            emit_attn_qb(0, i)
        emit_a2a(0)

        xt1_used = False
        ff_pool = None
        for i in range(N_QI):
            emit_qkv_tb(1, i)
            emit_attn_qb(1, i)
            if i == 2:
                # b0's exchange has landed by now; weave its FF norm into
                # the ACT-paced tail of b1's attention.
                ff_pool = ctx.enter_context(tc.tile_pool(name="ff_pool", bufs=1))
                ff_work = ctx.enter_context(tc.tile_pool(name="ff_work", bufs=1))
                ff_state["tiles"] = (
                    ff_pool.tile([2 * ECH, TOK_PC], f32, name="den16"),
                    ff_pool.tile([2 * ECH, TOK_PC], f32, name="recip16"),
                    ff_pool.tile([2 * ECH, TOK_PC], bf16, name="recip_bf"),
                    ff_pool.tile([P, ECH, TOK_PC], bf16, name="xres_bf"),
                )
                emit_ff_norm(0, ff_pool, ff_work)
        emit_a2a(1)
        emit_ff_mm(0, ff_work)
        emit_ff_norm(1, ff_pool, ff_work)
        emit_ff_mm(1, ff_work)



    nc.compile()
    return nc


def _np_bf16():
    import ml_dtypes

    return ml_dtypes.bfloat16


def _make_in_maps(x, Wq, Wk, Wv, Wf, bf, seq_t: int = T, n_cores: int = NCORES):
    bf16 = _np_bf16()
    t_all = B * T
    xT = np.ascontiguousarray(x.reshape(t_all, E).T).astype(bf16)  # [E, B*T]
    tri = np.triu(np.ones((P, P), np.float32)).astype(bf16)  # qi >= kj
    ident = np.eye(P, dtype=np.float32).astype(bf16)
    # ind row i = r*8 + s maps to head 2s+r (matching the den16 DMA order):
    # head k covers feature block ec=k//2, half (k%2) of its 128 cols.
    ind = np.zeros((2 * ECH, ECH, P), np.float32)
    for i in range(2 * ECH):
        r, s = i // ECH, i % ECH
        k = 2 * s + r
        ec, half = k // 2, k % 2
        ind[i, ec, half * DH : (half + 1) * DH] = 1.0
    ind = ind.astype(bf16)
    bfcol = np.ascontiguousarray(bf.reshape(E, 1)).astype(np.float32)
    wf_b = np.ascontiguousarray(Wf).astype(bf16)
    in_maps = []
    for c in range(n_cores):
        lo, hi = c * P, (c + 1) * P  # this core's head-pair feature cols
        wqkv_c = np.ascontiguousarray(
            np.concatenate([Wq[:, lo:hi], Wk[:, lo:hi], Wv[:, lo:hi]], axis=1)
        ).astype(bf16)
        xslice = np.concatenate(
            [
                xT[:, b * T + c * TPH : b * T + (c + 1) * TPH]
                for b in range(B)
            ],
            axis=1,
        )
        in_maps.append(
            {
                "xT": xT,
                "xsliceT": np.ascontiguousarray(xslice),
                "wqkv": wqkv_c,
                "wf": wf_b,
                "bfcol": bfcol,
                "tri": tri,
                "ident": ident,
                "indmat": ind,
            }
        )
    return in_maps


_BASS_CACHE = {}


def _get_bass(seq_t: int = T, n_cores: int = NCORES):
    key = (seq_t, n_cores)
    if key not in _BASS_CACHE:
        _BASS_CACHE[key] = _build_bass(n_cores)
    return _BASS_CACHE[key]


def _assemble(results, seq_t: int = T, n_cores: int = NCORES):
    outT = np.empty((E, B * T), np.float32)
    for c in range(n_cores):
        for b in range(B):
            outT[:, b * T + c * TPH : b * T + (c + 1) * TPH] = results[c][
                "outT"
            ][:, b * TPH : (b + 1) * TPH]
    return np.ascontiguousarray(outT.T).reshape(B, T, E).astype(np.float32)


def kernel(x, Wq, Wk, Wv, Wf, bf):
    """Full-input / full-output entry point. Shards across 8 NeuronCores."""
    from concourse.bass_utils import run_bass_kernel_spmd

    nc = _get_bass(T, NCORES)
    in_maps = _make_in_maps(
        np.asarray(x, np.float32),
        np.asarray(Wq, np.float32),
        np.asarray(Wk, np.float32),
        np.asarray(Wv, np.float32),
        np.asarray(Wf, np.float32),
        np.asarray(bf, np.float32),
        T,
        NCORES,
    )
    results = run_bass_kernel_spmd(nc, in_maps, list(range(NCORES))).results
    return _assemble(results, T, NCORES)


# revision 14
# speedup vs baseline: 1.0781x; 1.0781x over previous
"""Trainium2 Bass kernel for a fused transformer block (attention + FF).

Computation (B=2, T=2048, E=1024, H=16 heads, dh=64):
    q,k,v = x@Wq, x@Wk, x@Wv          (per-head, no bias)
    scores = q k^T / sqrt(E)  (causal)
    attn   = softmax(scores) v
    x1     = x + attn                  (no out-projection)
    out    = x1 + relu(x1 @ Wf + bf)

Sharding across 8 NeuronCores:
  - Attention is head-parallel: core c owns heads {2c, 2c+1} (128 feature
    cols of q/k/v).  Each core computes qT/kT/vT for its heads over ALL
    tokens, then causal-softmax attention in a transposed (S^T) layout.
  - An AllToAll redistributes attn^T from head-sharded to token-sharded.
  - FF is token-parallel: core c owns B*T/8 tokens of the flattened [B*T].

Performance structure (v2):
  - All inputs are pre-cast to bf16 on the host (halves HBM traffic).
  - xT is loaded in 512-token slabs alternating between the HWDGE (sync)
    and SWDGE (gpsimd) DMA queues so QKV matmuls start ~6us in.
  - Scores for BOTH heads go into one [128, 1024] PSUM tile (2 banks) as
    two concurrent K=64 matmuls row-packed via tile_position (0,0)/(64,0).
    st tiles are double-buffered (4 banks) so exp (ScalarE) overlaps the
    next score matmuls; AV matmuls are software-pipelined one kc behind.
  - Only the 128-wide diagonal block needs the triangular mask; columns
    left of it are memset and skipped by exp.
  - v natural layout comes from vT (N=512 matmuls) + PE-transpose chunks.
  - Softmax normalization happens on the FF side after the AllToAll; the
    denominators ride along as v's appended ones-column (row 64 of AV).
  - Batch 1's QKV work is emitted interleaved into batch 0's attention so
    the PE stays busy while the scalar engine works through exp calls.
"""

import math

import numpy as np

B, T, E, H = 2, 2048, 1024, 16
DH = E // H
NCORES = 8
P = 128  # partitions
QI_BLK = 512  # qi block width (one PSUM bank of fp32)
ECH = E // P  # feature chunks of 128
TOK_PC = B * T // NCORES  # tokens per core (FF phase)
TPH = TOK_PC // B  # tokens per core per batch
N_KC = T // P  # kj chunks per batch
N_QI = T // QI_BLK


def _build_bass(n_cores: int = NCORES):
    from contextlib import ExitStack

    import concourse.bacc as bacc
    import concourse.mybir as mybir
    import concourse.tile as tile

    t_all = B * T

    nc = bacc.Bacc(
        "TRN2",
        target_bir_lowering=False,
        debug=False,
        num_devices=n_cores,
    )

    dt = mybir.dt
    f32, bf16 = dt.float32, dt.bfloat16
    AF = mybir.ActivationFunctionType

    # ---- I/O (all matmul operands pre-cast to bf16 on the host) --------
    xT = nc.dram_tensor("xT", [E, t_all], bf16, kind="ExternalInput")
    xsliceT = nc.dram_tensor("xsliceT", [E, TOK_PC], bf16, kind="ExternalInput")
    wqkv = nc.dram_tensor("wqkv", [E, 3 * P], bf16, kind="ExternalInput")
    wf = nc.dram_tensor("wf", [E, E], bf16, kind="ExternalInput")
    bfcol = nc.dram_tensor("bfcol", [E, 1], f32, kind="ExternalInput")
    tri = nc.dram_tensor("tri", [P, P], bf16, kind="ExternalInput")
    ident = nc.dram_tensor("ident", [P, P], bf16, kind="ExternalInput")
    indmat = nc.dram_tensor("indmat", [2 * ECH, ECH, P], bf16, kind="ExternalInput")
    outT = nc.dram_tensor("outT", [E, TOK_PC], f32, kind="ExternalOutput")

    # Internal DRAM: per-batch all-to-all buffers
    # ([128 attn rows | 2 den rows] per shard); one exchange per batch so
    # batch 0's exchange overlaps batch 1's attention compute.
    a2a_in = [
        nc.dram_tensor(f"a2a_in{b}", [n_cores, P + 2, TPH], bf16)
        for b in range(B)
    ]
    a2a_out = [
        nc.dram_tensor(f"a2a_out{b}", [n_cores, P + 2, TPH], bf16)
        for b in range(B)
    ]
    bar_in = nc.dram_tensor("bar_in", [n_cores, 8], f32)
    bar_out = nc.dram_tensor("bar_out", [n_cores, 8], f32)

    scale = 1.0 / math.sqrt(E)

    with tile.TileContext(nc) as tc, ExitStack() as ctx:
        # ---- persistent SBUF -------------------------------------------
        persist = ctx.enter_context(tc.tile_pool(name="persist", bufs=1))
        wqkv_sb = persist.tile([P, ECH, 3 * P], bf16)
        wf_sb = persist.tile([P, ECH, ECH, P], bf16)
        bf_sb = persist.tile([P, ECH], f32)
        tri_sb = persist.tile([P, P], bf16)
        id_sb = persist.tile([P, P], bf16)
        ind_sb = persist.tile([2 * ECH, ECH, P], bf16)
        xsl_sb = persist.tile([P, ECH, TOK_PC], bf16)

        attn_pool = ctx.enter_context(tc.tile_pool(name="attn_pool", bufs=1))
        attnT = [
            attn_pool.tile([P, T], bf16, name=f"attnT{b}", tag=f"attnT{b}")
            for b in range(B)
        ]
        # dens at partitions 0 and 32 (engine writes must be 32-aligned)
        denT = [
            attn_pool.tile([33, T], bf16, name=f"denT{b}", tag=f"denT{b}")
            for b in range(B)
        ]

        # ---- PSUM pools: ps(2) + st(2x2) + av0/av1(2) = 8 banks --------
        ps_pool = ctx.enter_context(tc.tile_pool(name="ps", bufs=2, space="PSUM"))
        st_pool = ctx.enter_context(tc.tile_pool(name="st", bufs=2, space="PSUM"))
        av_pool = ctx.enter_context(tc.tile_pool(name="av", bufs=1, space="PSUM"))

        qkv_pool = ctx.enter_context(tc.tile_pool(name="qkv_pool", bufs=2))
        pt_pool = ctx.enter_context(tc.tile_pool(name="pt_pool", bufs=3))

        xt_pool = ctx.enter_context(tc.tile_pool(name="xt_pool", bufs=1))
        xT_sb = xt_pool.tile([P, ECH, t_all], bf16)

        # ---- DMA schedule: xT slabs first, FF-phase tensors later ------
        xT_r = xT.rearrange("(c p) t -> p c t", p=P)
        nc.sync.dma_start(out=wqkv_sb, in_=wqkv.rearrange("(c p) m -> p c m", p=P))
        for s in range(t_all // QI_BLK):
            sl = slice(s * QI_BLK, (s + 1) * QI_BLK)
            nc.sync.dma_start(out=xT_sb[:, :, sl], in_=xT_r[:, :, sl])
            if s == 0:
                nc.sync.dma_start(out=tri_sb, in_=tri[:, :])
                nc.sync.dma_start(out=id_sb, in_=ident[:, :])
        nc.sync.dma_start(out=bf_sb, in_=bfcol.rearrange("(c p) 1 -> p c", p=P))
        nc.sync.dma_start(
            out=xsl_sb, in_=xsliceT.rearrange("(c p) t -> p c t", p=P)
        )
        nc.gpsimd.dma_start(
            out=wf_sb, in_=wf.rearrange("(c p) (f m) -> p c f m", p=P, m=P)
        )
        nc.gpsimd.dma_start(out=ind_sb, in_=indmat[:, :, :])
        # Tiny barrier collective: absorbs cross-core start skew on the
        # gpsimd stream (compute continues) so the real exchanges below
        # never stall mid-transfer waiting for a late peer.
        nc.gpsimd.collective_compute(
            "AllToAll",
            mybir.AluOpType.bypass,
            replica_groups=[list(range(n_cores))],
            ins=[bar_in[:].opt()],
            outs=[bar_out[:].opt()],
        )

        qkvs = {}

        def emit_qkv_tb(b, tb):
            """QKV projection chains + v transposes for one 512-token block."""
            qT, kT, vT, v_sb = qkvs[b]
            t0 = b * T
            for which, dst in ((0, qT), (1, kT), (2, vT)):
                ps = ps_pool.tile([P, QI_BLK], f32, name="qk_ps", tag="ps")
                for ec in range(ECH):
                    nc.tensor.matmul(
                        ps,
                        lhsT=wqkv_sb[:, ec, which * P : (which + 1) * P],
                        rhs=xT_sb[
                            :, ec, t0 + tb * QI_BLK : t0 + (tb + 1) * QI_BLK
                        ],
                        start=(ec == 0),
                        stop=(ec == ECH - 1),
                    )
                nc.vector.tensor_copy(
                    dst[:, tb * QI_BLK : (tb + 1) * QI_BLK], ps
                )
            # v natural layout [token, head, dh+1] via PE transpose
            for kc in range(tb * 4, tb * 4 + 4):
                tp = ps_pool.tile([P, P], bf16, name="tp_ps", tag="ps")
                nc.tensor.transpose(tp, vT[:, kc * P : (kc + 1) * P], id_sb)
                nc.vector.tensor_copy(
                    v_sb[:, kc, :, 0:DH], tp.rearrange("p (h d) -> p h d", h=2)
                )
                nc.vector.memset(v_sb[:, kc, :, DH : DH + 1], 1.0)

        def emit_attn_qb(b, qb):
            """Causal attention for one 512-wide query block, both heads."""
            qT, kT, vT, v_sb = qkvs[b]
            last = 4 * qb + 3
            qsl = slice(qb * QI_BLK, (qb + 1) * QI_BLK)
            av0 = av_pool.tile([DH + 1, QI_BLK], f32, name="av0", tag="av0")
            av1 = av_pool.tile([DH + 1, QI_BLK], f32, name="av1", tag="av1")
            pending = None
            for kc in range(last + 1):
                st = st_pool.tile([P, 2 * QI_BLK], f32, name="st", tag="st")
                nc.tensor.matmul(
                    st[:, 0:QI_BLK],
                    lhsT=kT[0:DH, kc * P : (kc + 1) * P],
                    rhs=qT[0:DH, qsl],
                    start=True,
                    stop=True,
                )
                nc.tensor.matmul(
                    st[:, QI_BLK : 2 * QI_BLK],
                    lhsT=kT[DH:P, kc * P : (kc + 1) * P],
                    rhs=qT[DH:P, qsl],
                    start=True,
                    stop=True,
                )
                pt = pt_pool.tile([P, 2 * QI_BLK], bf16, name="pt", tag="pt")
                diag = kc >= 4 * qb
                c0 = (kc - 4 * qb) * P if diag else 0
                if c0 == 0:
                    nc.scalar.activation(pt, st, AF.Exp, scale=scale)
                else:
                    # one 3D-AP instruction covers both heads' regions
                    ptv = pt.rearrange("p (h q) -> p h q", h=2)
                    stv = st.rearrange("p (h q) -> p h q", h=2)
                    nc.vector.memset(ptv[:, :, 0:c0], 0.0)
                    nc.scalar.activation(
                        ptv[:, :, c0:QI_BLK],
                        stv[:, :, c0:QI_BLK],
                        AF.Exp,
                        scale=scale,
                    )
                if diag:
                    nc.vector.tensor_mul(
                        pt[:, c0 : c0 + P], pt[:, c0 : c0 + P], tri_sb
                    )
                    nc.vector.tensor_mul(
                        pt[:, QI_BLK + c0 : QI_BLK + c0 + P],
                        pt[:, QI_BLK + c0 : QI_BLK + c0 + P],
                        tri_sb,
                    )
                if pending is not None:
                    pending()
                kc_ = kc

                def do_av(kc=kc_, pt=pt):
                    nc.tensor.matmul(
                        av0,
                        lhsT=v_sb[:, kc, 0, :],
                        rhs=pt[:, 0:QI_BLK],
                        start=(kc == 0),
                        stop=(kc == last),
                    )
                    nc.tensor.matmul(
                        av1,
                        lhsT=v_sb[:, kc, 1, :],
                        rhs=pt[:, QI_BLK : 2 * QI_BLK],
                        start=(kc == 0),
                        stop=(kc == last),
                    )

                pending = do_av
            pending()
            # spill unnormalized numerators + denominators
            nc.vector.tensor_copy(attnT[b][0:DH, qsl], av0[0:DH, :])
            nc.vector.tensor_copy(attnT[b][DH:P, qsl], av1[0:DH, :])
            nc.vector.tensor_copy(denT[b][0:1, qsl], av0[DH : DH + 1, :])
            nc.vector.tensor_copy(denT[b][32:33, qsl], av1[DH : DH + 1, :])

        def emit_a2a(b):
            nc.sync.dma_start(
                out=a2a_in[b].rearrange("s p t -> p s t")[0:P],
                in_=attnT[b].rearrange("p (s t) -> p s t", s=n_cores),
            )
            nc.gpsimd.dma_start(
                out=a2a_in[b].rearrange("s p t -> p s t")[P : P + 2],
                in_=denT[b][0:33:32, :].rearrange("p (s t) -> p s t", s=n_cores),
            )
            nc.gpsimd.collective_compute(
                "AllToAll",
                mybir.AluOpType.bypass,
                replica_groups=[list(range(n_cores))],
                ins=[a2a_in[b][:].opt()],
                outs=[a2a_out[b][:].opt()],
            )

        # ---- FF emitters (token-parallel: TPH tokens per batch) --------
        ff_state = {}

        def emit_ff_norm(bh, ff_pool, ff_work):
            """Softmax-normalize + residual for this bh's tokens."""
            den16, recip16, recip_bf, xres_bf = ff_state["tiles"]
            hsl = slice(bh * TPH, (bh + 1) * TPH)
            a2a_r = a2a_out[bh].rearrange("s p t -> p s t")
            nc.gpsimd.dma_start(out=den16[:, hsl], in_=a2a_r[P : P + 2])
            nc.vector.reciprocal(recip16[:, hsl], den16[:, hsl])
            nc.vector.tensor_copy(recip_bf[:, hsl], recip16[:, hsl])
            at_all = ff_work.tile([P, ECH, TPH], bf16, name="at_all", tag="at")
            nc.gpsimd.dma_start(out=at_all, in_=a2a_r[0:P])
            for ec in range(ECH):
                bc = ps_pool.tile([P, TPH], f32, name="bc_ps", tag="ps")
                nc.tensor.matmul(
                    bc,
                    lhsT=ind_sb[:, ec, :],
                    rhs=recip_bf[:, hsl],
                    start=True,
                    stop=True,
                )
                nc.vector.tensor_mul(at_all[:, ec, :], at_all[:, ec, :], bc)
                nc.vector.tensor_add(
                    xres_bf[:, ec, hsl], xsl_sb[:, ec, hsl], at_all[:, ec, :]
                )

        def emit_ff_mm(bh, ff_work):
            den16, recip16, recip_bf, xres_bf = ff_state["tiles"]
            hsl = slice(bh * TPH, (bh + 1) * TPH)
            o_all = ff_work.tile([P, ECH, TPH], f32, name="o_all", tag="o")
            for fc in range(ECH):
                ps = ps_pool.tile([P, TPH], f32, name="ff_ps", tag="ps")
                for ec in range(ECH):
                    nc.tensor.matmul(
                        ps,
                        lhsT=wf_sb[:, ec, fc, :],
                        rhs=xres_bf[:, ec, hsl],
                        start=(ec == 0),
                        stop=(ec == ECH - 1),
                    )
                relu = ff_work.tile([P, TPH], f32, name="relu", tag="relu")
                nc.scalar.activation(
                    relu, ps, AF.Relu, bias=bf_sb[:, fc : fc + 1]
                )
                nc.vector.tensor_add(o_all[:, fc, :], relu, xres_bf[:, fc, hsl])
                if fc % 4 == 3:
                    fsl = slice(fc - 3, fc + 1)
                    nc.sync.dma_start(
                        out=outT.rearrange("(c p) t -> p c t", p=P)[:, fsl, hsl],
                        in_=o_all[:, fsl, :],
                    )

        # ---- emission: qkv/attn interleaved; FF(bh0) woven into b1 -----
        for b in range(B):
            qkvs[b] = (
                qkv_pool.tile([P, T], bf16, name=f"qT{b}", tag="qT"),
                qkv_pool.tile([P, T], bf16, name=f"kT{b}", tag="kT"),
                qkv_pool.tile([P, T], bf16, name=f"vT{b}", tag="vT"),
                qkv_pool.tile([P, N_KC, 2, DH + 1], bf16, name=f"v{b}", tag="v"),
            )
        for i in range(N_QI):
            emit_qkv_tb(0, i)
            emit_attn_qb(0, i)
        emit_a2a(0)

        xt1_used = False
        ff_pool = None
        for i in range(N_QI):
            emit_qkv_tb(1, i)
            emit_attn_qb(1, i)
            if i == 2:
                # b0's exchange has landed by now; weave its FF norm into
                # the ACT-paced tail of b1's attention.
                ff_pool = ctx.enter_context(tc.tile_pool(name="ff_pool", bufs=1))
                ff_work = ctx.enter_context(tc.tile_pool(name="ff_work", bufs=1))
                ff_state["tiles"] = (
                    ff_pool.tile([2 * ECH, TOK_PC], bf16, name="den16"),
                    ff_pool.tile([2 * ECH, TOK_PC], f32, name="recip16"),
                    ff_pool.tile([2 * ECH, TOK_PC], bf16, name="recip_bf"),
                    ff_pool.tile([P, ECH, TOK_PC], bf16, name="xres_bf"),
                )
                emit_ff_norm(0, ff_pool, ff_work)
        emit_a2a(1)
        emit_ff_mm(0, ff_work)
        emit_ff_norm(1, ff_pool, ff_work)
        emit_ff_mm(1, ff_work)



    nc.compile()
    return nc


def _np_bf16():
    import ml_dtypes

    return ml_dtypes.bfloat16


def _make_in_maps(x, Wq, Wk, Wv, Wf, bf, seq_t: int = T, n_cores: int = NCORES):
    bf16 = _np_bf16()
    t_all = B * T
    xT = np.ascontiguousarray(x.reshape(t_all, E).T).astype(bf16)  # [E, B*T]
    tri = np.triu(np.ones((P, P), np.float32)).astype(bf16)  # qi >= kj
    ident = np.eye(P, dtype=np.float32).astype(bf16)
    # ind row i = r*8 + s maps to head 2s+r (matching the den16 DMA order):
    # head k covers feature block ec=k//2, half (k%2) of its 128 cols.
    ind = np.zeros((2 * ECH, ECH, P), np.float32)
    for i in range(2 * ECH):
        r, s = i // ECH, i % ECH
        k = 2 * s + r
        ec, half = k // 2, k % 2
        ind[i, ec, half * DH : (half + 1) * DH] = 1.0
    ind = ind.astype(bf16)
    bfcol = np.ascontiguousarray(bf.reshape(E, 1)).astype(np.float32)
    wf_b = np.ascontiguousarray(Wf).astype(bf16)
    in_maps = []
    for c in range(n_cores):
        lo, hi = c * P, (c + 1) * P  # this core's head-pair feature cols
        wqkv_c = np.ascontiguousarray(
            np.concatenate([Wq[:, lo:hi], Wk[:, lo:hi], Wv[:, lo:hi]], axis=1)
        ).astype(bf16)
        xslice = np.concatenate(
            [
                xT[:, b * T + c * TPH : b * T + (c + 1) * TPH]
                for b in range(B)
            ],
            axis=1,
        )
        in_maps.append(
            {
                "xT": xT,
                "xsliceT": np.ascontiguousarray(xslice),
                "wqkv": wqkv_c,
                "wf": wf_b,
                "bfcol": bfcol,
                "tri": tri,
                "ident": ident,
                "indmat": ind,
            }
        )
    return in_maps


_BASS_CACHE = {}


def _get_bass(seq_t: int = T, n_cores: int = NCORES):
    key = (seq_t, n_cores)
    if key not in _BASS_CACHE:
        _BASS_CACHE[key] = _build_bass(n_cores)
    return _BASS_CACHE[key]


def _assemble(results, seq_t: int = T, n_cores: int = NCORES):
    outT = np.empty((E, B * T), np.float32)
    for c in range(n_cores):
        for b in range(B):
            outT[:, b * T + c * TPH : b * T + (c + 1) * TPH] = results[c][
                "outT"
            ][:, b * TPH : (b + 1) * TPH]
    return np.ascontiguousarray(outT.T).reshape(B, T, E).astype(np.float32)


def kernel(x, Wq, Wk, Wv, Wf, bf):
    """Full-input / full-output entry point. Shards across 8 NeuronCores."""
    from concourse.bass_utils import run_bass_kernel_spmd

    nc = _get_bass(T, NCORES)
    in_maps = _make_in_maps(
        np.asarray(x, np.float32),
        np.asarray(Wq, np.float32),
        np.asarray(Wk, np.float32),
        np.asarray(Wv, np.float32),
        np.asarray(Wf, np.float32),
        np.asarray(bf, np.float32),
        T,
        NCORES,
    )
    results = run_bass_kernel_spmd(nc, in_maps, list(range(NCORES))).results
    return _assemble(results, T, NCORES)
